# revision 1
# baseline (speedup 1.0000x reference)
"""HGT (2-type, 2-relation, 2-layer) Bass kernel for 8 Trainium2 cores.

Sharding: destination-node sharding. Core c owns dst rows [5120c, 5120(c+1))
of both node types (core 7 partially padded). Each core builds the full
folded K'/V' source tables (projections replicated), gathers per-edge
source rows with dma_gather (int16 indices; src space split at 32768 into
lo/hi sub-tables), computes per-edge attention with one-hot matmuls per
128-edge block (segment softmax without max-subtraction), and accumulates
per 128-dst group in PSUM. New node features are AllGathered between the
two layers.
"""
import math
import os
import sys

import numpy as np

sys.path.insert(0, "/opt/trn_rl_repo")

H, D, C, L = 4, 32, 128, 2
INV_SQRT_D = 1.0 / math.sqrt(D)
P = 128
NCORES = 8
SHARD = 5120          # 40 groups of 128 dst rows per core
NGRP = SHARD // P     # 40
NPAD = NCORES * SHARD # 40960
LO_LIM = 32768
HI_ROWS = NPAD - LO_LIM
CHUNK_BLK = 12        # gather chunk size in 128-edge blocks

LAST_RESULT = None


def _fold_weights(ins):
    """Fold a_rel/m_rel into k/v weights, p_rel/sqrt(D) into q, sigmoid(skip)
    into a_lin. Returns dict of numpy arrays."""
    f = {}
    for l in range(L):
        for t in range(2):
            kw = np.asarray(ins["k_w"][l, t])   # [C, C]
            kb = np.asarray(ins["k_b"][l, t])   # [C]
            vw = np.asarray(ins["v_w"][l, t])
            vb = np.asarray(ins["v_b"][l, t])
            ar = np.asarray(ins["a_rel"][l, t])  # [H, D, D]; type t is src of rel t
            mr = np.asarray(ins["m_rel"][l, t])
            wk = np.zeros((C, C), np.float32)
            wv = np.zeros((C, C), np.float32)
            bk = np.zeros(C, np.float32)
            bv = np.zeros(C, np.float32)
            for h in range(H):
                sl = slice(h * D, (h + 1) * D)
                wk[:, sl] = kw[:, sl] @ ar[h]
                wv[:, sl] = vw[:, sl] @ mr[h]
                bk[sl] = kb[sl] @ ar[h]
                bv[sl] = vb[sl] @ mr[h]
            f[f"Wkv{l}{t}"] = np.concatenate([wk, wv], axis=1)          # [C, 2C]
            f[f"Bkv{l}{t}"] = np.tile(np.concatenate([bk, bv])[None, :], (P, 1))
            # q: type t is dst of relation (1 - t)
            r_dst = 1 - t
            pr = np.asarray(ins["p_rel"][l, r_dst]) * INV_SQRT_D        # [H]
            scale = np.repeat(pr, D)                                    # [C]
            f[f"Wq{l}{t}"] = (np.asarray(ins["q_w"][l, t]) * scale[None, :]).astype(np.float32)
            f[f"Bq{l}{t}"] = np.tile((np.asarray(ins["q_b"][l, t]) * scale)[None, :], (P, 1))
            s = 1.0 / (1.0 + math.exp(-float(np.asarray(ins["skip"][l, t]))))
            f[f"Wal{l}{t}"] = (np.asarray(ins["a_lin_w"][l, t]) * s).astype(np.float32)
            f[f"Bal{l}{t}"] = np.tile((np.asarray(ins["a_lin_b"][l, t]) * s)[None, :], (P, 1))
            f[f"oms{l}{t}"] = 1.0 - s
    f["Wina"] = np.asarray(ins["lin_a_w"]).astype(np.float32)
    f["Binb"] = np.tile(np.asarray(ins["lin_b_b"])[None, :], (P, 1)).astype(np.float32)
    f["Bina"] = np.tile(np.asarray(ins["lin_a_b"])[None, :], (P, 1)).astype(np.float32)
    f["Winb"] = np.asarray(ins["lin_b_w"]).astype(np.float32)
    return f


def _prep_edges(edge):
    """Partition one relation's edges by dst shard; build per-core gather
    index / dst_rel arrays plus the shared static block schedule.

    Returns (idx_w[8], dst_rel_t[8], nblk_lo[NGRP], nblk_hi[NGRP])."""
    src = np.asarray(edge[0]).astype(np.int64)
    dst = np.asarray(edge[1]).astype(np.int64)
    core = dst // SHARD
    per = []  # per core: list over groups of (lo_src, lo_rel, hi_src, hi_rel)
    nblk_lo = np.zeros(NGRP, np.int64)
    nblk_hi = np.zeros(NGRP, np.int64)
    for c in range(NCORES):
        m = core == c
        s, dl = src[m], dst[m] - c * SHARD
        g = dl // P
        rel = dl % P
        glists = []
        for gi in range(NGRP):
            gm = g == gi
            sg, rg = s[gm], rel[gm]
            lo = sg < LO_LIM
            glists.append((sg[lo], rg[lo], sg[~lo] - LO_LIM, rg[~lo]))
            nblk_lo[gi] = max(nblk_lo[gi], (len(sg[lo]) + P - 1) // P)
            nblk_hi[gi] = max(nblk_hi[gi], (len(sg[~lo]) + P - 1) // P)
    nblk_lo = np.maximum(nblk_lo, 1)
    TB = int(nblk_lo.sum() + nblk_hi.sum())
    SL = TB * P
    idx_ws, dr_ts = [], []
    for c in range(NCORES):
        m = core == c
        s, dl = src[m], dst[m] - c * SHARD
        g = dl // P
        rel = dl % P
        idx = np.zeros(SL, np.int16)
        dr = np.full(SL, -1.0, np.float32)
        pos = 0
        for gi in range(NGRP):
            gm = g == gi
            sg, rg = s[gm], rel[gm]
            lo = sg < LO_LIM
            sl_, rl_ = sg[lo], rg[lo]
            idx[pos:pos + len(sl_)] = sl_
            dr[pos:pos + len(sl_)] = rl_
            pos += int(nblk_lo[gi]) * P
        for gi in range(NGRP):
            gm = g == gi
            sg, rg = s[gm], rel[gm]
            hi = sg >= LO_LIM
            sh_, rh_ = sg[hi] - LO_LIM, rg[hi]
            idx[pos:pos + len(sh_)] = sh_
            dr[pos:pos + len(sh_)] = rh_
            pos += int(nblk_hi[gi]) * P
        assert pos == SL
        idx_ws.append(np.tile(idx.reshape(SL // 16, 16).T, (8, 1)).copy())
        dr_ts.append(dr.reshape(TB, P).T.copy())
    return idx_ws, dr_ts, nblk_lo.tolist(), nblk_hi.tolist()


def _chunks(nblk_per_grp):
    """Split the per-group block list of one region into gather chunks of up
    to CHUNK_BLK blocks. Returns (chunk list [(blk_start, nblk)],
    per-group [(chunk_i, local_b)])."""
    total = sum(nblk_per_grp)
    chunks = []
    b = 0
    while b < total:
        n = min(CHUNK_BLK, total - b)
        chunks.append((b, n))
        b += n
    loc = []
    b = 0
    for g, n in enumerate(nblk_per_grp):
        lst = []
        for i in range(n):
            blk = b + i
            ci = blk // CHUNK_BLK
            lst.append((ci, blk - chunks[ci][0]))
        loc.append(lst)
        b += n
    return chunks, loc


def kernel(**ins):
    global LAST_RESULT
    import concourse.bass as bass
    import concourse.tile as tile
    from concourse import bacc, mybir
    from concourse.bass_utils import run_bass_kernel_spmd
    from concourse.masks import make_identity

    FP = mybir.dt.float32
    I16 = mybir.dt.int16
    I32 = mybir.dt.int32
    AL = mybir.AluOpType
    AF = mybir.ActivationFunctionType

    f = _fold_weights(ins)
    idx0, dr0, nlo0, nhi0 = _prep_edges(np.asarray(ins["edge_ab"]))
    idx1, dr1, nlo1, nhi1 = _prep_edges(np.asarray(ins["edge_ba"]))
    rel_meta = [(nlo0, nhi0), (nlo1, nhi1)]
    TBs = [sum(nlo0) + sum(nhi0), sum(nlo1) + sum(nhi1)]

    xa = np.asarray(ins["x_a"]).astype(np.float32)
    xb = np.asarray(ins["x_b"]).astype(np.float32)
    DA, DB = xa.shape[1], xb.shape[1]
    xaT = np.zeros((DA, NPAD), np.float32)
    xaT[:, :40000] = xa.T
    xbT = np.zeros((DB, NPAD), np.float32)
    xbT[:, :40000] = xb.T

    nc = bacc.Bacc("TRN2", target_bir_lowering=False, debug=False, num_devices=NCORES)

    # ---- DRAM tensors ----
    t_xaT = nc.dram_tensor("xaT", [DA, NPAD], FP, kind="ExternalInput").ap()
    t_xbT = nc.dram_tensor("xbT", [DB, NPAD], FP, kind="ExternalInput").ap()
    t_xasT = nc.dram_tensor("xasT", [DA, SHARD], FP, kind="ExternalInput").ap()
    t_xbsT = nc.dram_tensor("xbsT", [DB, SHARD], FP, kind="ExternalInput").ap()
    wnames = ["Wina", "Winb", "Bina", "Binb"]
    for l in range(L):
        for t in range(2):
            wnames += [f"Wkv{l}{t}", f"Bkv{l}{t}", f"Wq{l}{t}", f"Bq{l}{t}",
                       f"Wal{l}{t}", f"Bal{l}{t}"]
    t_w = {n: nc.dram_tensor(n, list(f[n].shape), FP, kind="ExternalInput").ap()
           for n in wnames}
    t_idx = [nc.dram_tensor(f"idx{r}", [P, TBs[r] * 8], I16, kind="ExternalInput").ap()
             for r in range(2)]
    t_dr = [nc.dram_tensor(f"dr{r}", [P, TBs[r]], FP, kind="ExternalInput").ap()
            for r in range(2)]

    t_kv = [nc.dram_tensor(f"kv{t}", [NPAD, 2 * C], FP) for t in range(2)]
    t_x0s = [nc.dram_tensor(f"x0s{t}", [SHARD, C], FP) for t in range(2)]
    t_x1s = [nc.dram_tensor(f"x1s{t}", [SHARD, C], FP) for t in range(2)]
    t_nxT = [nc.dram_tensor(f"nxT{t}", [P, SHARD], FP) for t in range(2)]
    t_ag = [nc.dram_tensor(f"ag{t}", [NCORES, P, SHARD], FP, addr_space="Shared")
            for t in range(2)]
    t_out = [nc.dram_tensor(f"out{t}", [SHARD, C], FP, kind="ExternalOutput").ap()
             for t in range(2)]

    with tile.TileContext(nc) as tc:
        cpool_cm = tc.tile_pool(name="const", bufs=1)
        cpool = cpool_cm.__enter__()
        ident = cpool.tile([P, P], FP)
        make_identity(nc, ident[:])
        ioi = cpool.tile([P, P], I32)
        nc.gpsimd.iota(ioi[:], pattern=[[1, P]], base=0, channel_multiplier=0)
        iota_row = cpool.tile([P, P], FP)
        nc.vector.tensor_copy(iota_row[:], ioi[:])
        ioc = cpool.tile([P, 1], I32)
        nc.gpsimd.iota(ioc[:], pattern=[[0, 1]], base=0, channel_multiplier=1)
        iota_col = cpool.tile([P, 1], FP)
        nc.vector.tensor_copy(iota_col[:], ioc[:])
        w_sb = {}
        for n in wnames:
            w_sb[n] = cpool.tile(list(f[n].shape), FP, name=n, tag=n)
            nc.sync.dma_start(out=w_sb[n][:], in_=t_w[n][:])
        dr_sb = []
        for r in range(2):
            drt = cpool.tile([P, TBs[r]], FP, name=f"drsb{r}", tag=f"drsb{r}")
            nc.sync.dma_start(out=drt[:], in_=t_dr[r][:])
            dr_sb.append(drt)
        idx_sb = []
        for r in range(2):
            it = cpool.tile([P, TBs[r] * 8], I16, name=f"idxsb{r}", tag=f"idxsb{r}")
            nc.sync.dma_start(out=it[:], in_=t_idx[r][:])
            idx_sb.append(it)
        q_sb = [cpool.tile([P, NGRP, C], FP, name=f"qsb{t}", tag=f"qsb{t}") for t in range(2)]
        acc_sb = cpool.tile([P, NGRP, 132], FP)

        # ---------- layer-0 full tables (two-stage projection) ----------
        def input_proj(xT_ap, Win, Bin, t, j, src_pool, ps_pool, out_pool):
            DIN = xT_ap.shape[0]
            lhs = src_pool.tile([DIN, P], FP, tag="lhs0")
            nc.sync.dma_start(out=lhs[:], in_=xT_ap[:, j * P:(j + 1) * P])
            ps1 = ps_pool.tile([P, C], FP, space="PSUM", tag="ps1")
            nc.tensor.matmul(out=ps1[:], lhsT=lhs[:], rhs=w_sb[Win][:], start=True, stop=True)
            x0 = out_pool.tile([P, C], FP, tag="x0")
            nc.vector.tensor_tensor(out=x0[:], in0=ps1[:], in1=w_sb[Bin][:], op=AL.add)
            x0r = out_pool.tile([P, C], FP, tag="x0r")
            nc.scalar.activation(out=x0r[:], in_=x0[:], func=AF.Relu)
            pst = ps_pool.tile([P, P], FP, space="PSUM", tag="pst0")
            nc.tensor.transpose(out=pst[:], in_=x0r[:], identity=ident[:])
            x0T = out_pool.tile([P, P], FP, tag="x0T")
            nc.vector.tensor_copy(x0T[:], pst[:])
            return x0r, x0T

        with (
            tc.tile_pool(name="p0src", bufs=3) as src_pool,
            tc.tile_pool(name="p0ps", bufs=2, space="PSUM") as ps_pool,
            tc.tile_pool(name="p0out", bufs=3) as out_pool,
        ):
            for t, (xT_ap, Win, Bin) in enumerate(
                [(t_xaT, "Wina", "Bina"), (t_xbT, "Winb", "Binb")]
            ):
                for j in range(NPAD // P):
                    _, x0T = input_proj(xT_ap, Win, Bin, t, j, src_pool, ps_pool, out_pool)
                    ps2 = ps_pool.tile([P, 2 * C], FP, space="PSUM", tag="ps2")
                    nc.tensor.matmul(out=ps2[:], lhsT=x0T[:], rhs=w_sb[f"Wkv0{t}"][:],
                                     start=True, stop=True)
                    kvt = out_pool.tile([P, 2 * C], FP, tag="kvt")
                    nc.vector.tensor_tensor(out=kvt[:], in0=ps2[:], in1=w_sb[f"Bkv0{t}"][:], op=AL.add)
                    nc.sync.dma_start(out=t_kv[t].ap()[j * P:(j + 1) * P, :], in_=kvt[:])
            # shard pass: x0 shard rm + q0
            for t, (xsT_ap, Win, Bin) in enumerate(
                [(t_xasT, "Wina", "Bina"), (t_xbsT, "Winb", "Binb")]
            ):
                for j in range(NGRP):
                    x0r, x0T = input_proj(xsT_ap, Win, Bin, t, j, src_pool, ps_pool, out_pool)
                    nc.sync.dma_start(out=t_x0s[t].ap()[j * P:(j + 1) * P, :], in_=x0r[:])
                    psq = ps_pool.tile([P, C], FP, space="PSUM", tag="psq")
                    nc.tensor.matmul(out=psq[:], lhsT=x0T[:], rhs=w_sb[f"Wq0{t}"][:],
                                     start=True, stop=True)
                    nc.vector.tensor_tensor(out=q_sb[t][:, j, :], in0=psq[:],
                                            in1=w_sb[f"Bq0{t}"][:], op=AL.add)

        # ---------- per-layer processing ----------
        def attention(r, l):
            """relation r: src type = r, dst type = 1 - r. Fills acc_sb."""
            nlo, nhi = rel_meta[r]
            lo_chunks, lo_loc = _chunks(nlo)
            hi_chunks, hi_loc = _chunks(nhi)
            lo_base = 0
            hi_base = sum(nlo)
            kv_ap = t_kv[r].ap()
            qt = q_sb[1 - r]
            drt = dr_sb[r]
            idxt = idx_sb[r]
            with (
                tc.tile_pool(name=f"gat{r}{l}", bufs=2) as gpool,
                tc.tile_pool(name=f"aps{r}{l}", bufs=2, space="PSUM") as aps,
                tc.tile_pool(name=f"accp{r}{l}", bufs=2, space="PSUM") as accp,
                tc.tile_pool(name=f"asb{r}{l}", bufs=3) as asb,
            ):
                tiles = {}

                def get_chunk(region, ci):
                    key = (region, ci)
                    if key in tiles:
                        return tiles[key]
                    chunks = lo_chunks if region == 0 else hi_chunks
                    base = lo_base if region == 0 else hi_base
                    b0, n = chunks[ci]
                    gt = gpool.tile([P, CHUNK_BLK, 2 * C], FP, tag="kvchunk")
                    in_ap = kv_ap[0:LO_LIM, :] if region == 0 else kv_ap[LO_LIM:NPAD, :]
                    if os.environ.get("SKIP_GATHER"):
                        nc.vector.memset(gt[:, 0:n, :], 1.0)
                    else:
                        nc.gpsimd.dma_gather(
                            out_ap=gt[:, 0:n, :], in_ap=in_ap,
                            idxs_ap=idxt[:, (base + b0) * 8:(base + b0 + n) * 8],
                            num_idxs=n * P, num_idxs_reg=n * P, elem_size=2 * C,
                            single_packet=False,
                        )
                    tiles[key] = gt
                    return gt

                for g in range(NGRP):
                    blks = []
                    for i, (ci, lb) in enumerate(lo_loc[g]):
                        gb = lo_base + sum(nlo[:g]) + i
                        blks.append((0, ci, lb, gb))
                    for i, (ci, lb) in enumerate(hi_loc[g]):
                        gb = hi_base + sum(nhi[:g]) + i
                        blks.append((1, ci, lb, gb))
                    accps = accp.tile([P, 132], FP, space="PSUM", tag="acc")
                    for bi, (region, ci, lb, gb) in enumerate(blks):
                        gt = get_chunk(region, ci)
                        dcol = drt[:, gb:gb + 1]
                        oh = asb.tile([P, P], FP, tag="oh")
                        nc.vector.tensor_scalar(out=oh[:], in0=iota_row[:], scalar1=dcol,
                                                scalar2=None, op0=AL.is_equal)
                        pst = aps.tile([P, P], FP, space="PSUM", tag="pst")
                        nc.tensor.transpose(out=pst[:], in_=dcol.to_broadcast([P, P]),
                                            identity=ident[:])
                        ohT = asb.tile([P, P], FP, tag="ohT")
                        nc.vector.tensor_scalar(out=ohT[:], in0=pst[:], scalar1=iota_col[:],
                                                scalar2=None, op0=AL.is_equal)
                        qg = aps.tile([P, P], FP, space="PSUM", tag="qg")
                        nc.tensor.matmul(out=qg[:], lhsT=ohT[:], rhs=qt[:, g, :],
                                         start=True, stop=True)
                        lp = asb.tile([P, P], FP, tag="lp")
                        nc.vector.tensor_tensor(out=lp[:], in0=qg[:], in1=gt[:, lb, 0:C],
                                                op=AL.mult)
                        z = asb.tile([P, H], FP, tag="z")
                        nc.vector.tensor_reduce(out=z[:], in_=lp[:].rearrange(
                            "p (h d) -> p h d", h=H), axis=mybir.AxisListType.X, op=AL.add)
                        ze = asb.tile([P, H], FP, tag="ze")
                        nc.scalar.activation(out=ze[:], in_=z[:], func=AF.Exp)
                        wz = asb.tile([P, 132], FP, tag="wz")
                        nc.vector.tensor_tensor(
                            out=wz[:, 0:C], in0=gt[:, lb, C:2 * C],
                            in1=ze[:].rearrange("p (h o) -> p h o", o=1).to_broadcast([P, H, D]),
                            op=AL.mult)
                        nc.vector.tensor_copy(wz[:, C:132], ze[:])
                        nc.tensor.matmul(out=accps[:], lhsT=oh[:], rhs=wz[:],
                                         start=(bi == 0), stop=(bi == len(blks) - 1))
                    nc.vector.tensor_copy(acc_sb[:, g, :], accps[:])

        def alin(t, l):
            """a_lin + skip for dst type t of layer l; reads acc_sb."""
            xprev = t_x0s[t] if l == 0 else t_x1s[t]
            with (
                tc.tile_pool(name=f"al{t}{l}", bufs=3) as sp,
                tc.tile_pool(name=f"alp{t}{l}", bufs=2, space="PSUM") as pp,
            ):
                for j in range(NGRP):
                    den = sp.tile([P, H], FP, tag="den")
                    nc.vector.tensor_scalar(out=den[:], in0=acc_sb[:, j, C:132],
                                            scalar1=1e-16, scalar2=None, op0=AL.add)
                    rec = sp.tile([P, H], FP, tag="rec")
                    nc.vector.reciprocal(rec[:], den[:])
                    at = sp.tile([P, C], FP, tag="at")
                    nc.vector.tensor_tensor(
                        out=at[:], in0=acc_sb[:, j, 0:C],
                        in1=rec[:].rearrange("p (h o) -> p h o", o=1).to_broadcast([P, H, D]),
                        op=AL.mult)
                    gl = sp.tile([P, C], FP, tag="gl")
                    nc.scalar.activation(out=gl[:], in_=at[:], func=AF.Gelu)
                    pst = pp.tile([P, P], FP, space="PSUM", tag="apst")
                    nc.tensor.transpose(out=pst[:], in_=gl[:], identity=ident[:])
                    glT = sp.tile([P, P], FP, tag="glT")
                    nc.vector.tensor_copy(glT[:], pst[:])
                    pso = pp.tile([P, C], FP, space="PSUM", tag="pso")
                    nc.tensor.matmul(out=pso[:], lhsT=glT[:], rhs=w_sb[f"Wal{l}{t}"][:],
                                     start=True, stop=True)
                    o1 = sp.tile([P, C], FP, tag="o1")
                    nc.vector.tensor_tensor(out=o1[:], in0=pso[:], in1=w_sb[f"Bal{l}{t}"][:],
                                            op=AL.add)
                    xp = sp.tile([P, C], FP, tag="xp")
                    nc.sync.dma_start(out=xp[:], in_=xprev.ap()[j * P:(j + 1) * P, :])
                    o2 = sp.tile([P, C], FP, tag="o2")
                    nc.vector.tensor_scalar(out=o2[:], in0=xp[:], scalar1=f[f"oms{l}{t}"],
                                            scalar2=None, op0=AL.mult)
                    nw = sp.tile([P, C], FP, tag="nw")
                    nc.vector.tensor_tensor(out=nw[:], in0=o1[:], in1=o2[:], op=AL.add)
                    if l == 0:
                        nc.sync.dma_start(out=t_x1s[t].ap()[j * P:(j + 1) * P, :], in_=nw[:])
                        pst2 = pp.tile([P, P], FP, space="PSUM", tag="apst2")
                        nc.tensor.transpose(out=pst2[:], in_=nw[:], identity=ident[:])
                        nwT = sp.tile([P, P], FP, tag="nwT")
                        nc.vector.tensor_copy(nwT[:], pst2[:])
                        nc.sync.dma_start(out=t_nxT[t].ap()[:, j * P:(j + 1) * P], in_=nwT[:])
                        psq = pp.tile([P, C], FP, space="PSUM", tag="apsq")
                        nc.tensor.matmul(out=psq[:], lhsT=nwT[:], rhs=w_sb[f"Wq1{t}"][:],
                                         start=True, stop=True)
                        nc.vector.tensor_tensor(out=q_sb[t][:, j, :], in0=psq[:],
                                                in1=w_sb[f"Bq1{t}"][:], op=AL.add)
                    else:
                        nc.sync.dma_start(out=t_out[t][j * P:(j + 1) * P, :], in_=nw[:])
                if l == 0:
                    if os.environ.get("SKIP_AG"):
                        for k in range(NCORES):
                            nc.sync.dma_start(out=t_ag[t].ap()[k, :, :], in_=t_nxT[t].ap()[:])
                    else:
                        nc.gpsimd.collective_compute(
                            "AllGather", mybir.AluOpType.bypass,
                            replica_groups=[list(range(NCORES))],
                            ins=[t_nxT[t].ap()[:]], outs=[t_ag[t].ap()[:]],
                        )

        # layer 0 attention + alin (+ AllGather inside alin)
        attention(0, 0)
        alin(1, 0)
        attention(1, 0)
        alin(0, 0)

        # layer-1 kv tables from AllGather output
        with (
            tc.tile_pool(name="p1src", bufs=3) as src_pool,
            tc.tile_pool(name="p1ps", bufs=2, space="PSUM") as ps_pool,
            tc.tile_pool(name="p1out", bufs=3) as out_pool,
        ):
            for t in range(2):
                for k in range(NCORES):
                    for j in range(NGRP):
                        lhs = src_pool.tile([P, P], FP, tag="lhs1")
                        nc.sync.dma_start(out=lhs[:], in_=t_ag[t].ap()[k, :, j * P:(j + 1) * P])
                        ps2 = ps_pool.tile([P, 2 * C], FP, space="PSUM", tag="ps2")
                        nc.tensor.matmul(out=ps2[:], lhsT=lhs[:], rhs=w_sb[f"Wkv1{t}"][:],
                                         start=True, stop=True)
                        kvt = out_pool.tile([P, 2 * C], FP, tag="kvt")
                        nc.vector.tensor_tensor(out=kvt[:], in0=ps2[:],
                                                in1=w_sb[f"Bkv1{t}"][:], op=AL.add)
                        row = k * SHARD + j * P
                        nc.sync.dma_start(out=t_kv[t].ap()[row:row + P, :], in_=kvt[:])

        attention(0, 1)
        alin(1, 1)
        attention(1, 1)
        alin(0, 1)
        cpool_cm.__exit__(None, None, None)

    nc.compile()

    in_maps = []
    for c in range(NCORES):
        m = {"xaT": xaT, "xbT": xbT,
             "xasT": np.ascontiguousarray(xaT[:, c * SHARD:(c + 1) * SHARD]),
             "xbsT": np.ascontiguousarray(xbT[:, c * SHARD:(c + 1) * SHARD]),
             "idx0": idx0[c], "dr0": dr0[c], "idx1": idx1[c], "dr1": dr1[c]}
        for n in wnames:
            m[n] = np.ascontiguousarray(f[n])
        in_maps.append(m)

    res = run_bass_kernel_spmd(
        nc, in_maps, core_ids=list(range(NCORES)),
        trace=bool(os.environ.get("BASS_TRACE")),
    )
    LAST_RESULT = res
    outa = np.concatenate([res.results[c]["out0"] for c in range(NCORES)])[:40000]
    outb = np.concatenate([res.results[c]["out1"] for c in range(NCORES)])[:40000]
    return outa, outb



# revision 17
# speedup vs baseline: 3.1268x; 3.1268x over previous
"""HGT (2-type, 2-relation, 2-layer) Bass kernel for 8 Trainium2 cores — v2.

Design vs v0 baseline (6.78 ms):
- bf16 for all matmul operands / tables / gathers (PE 4x, gather bytes 2x).
- x kept in TRANSPOSED layout [c, node] end-to-end: input proj emits x0T
  directly (lhsT=Win), no PE transposes outside alin's gelu output.
- kv tables built per-shard only (40 groups/core) and AllGathered in bf16;
  kills the 640-block full-table rebuilds of v0.
- Host-side LPT permutation balances dst-group degree; two overlapping
  int16 gather windows ([0,32K) and [8K,40K)) let flex edges fill lo blocks
  exactly: 4 lo + 3 hi blocks per group (TB=280/rel vs 340).
- Attention per 128-edge block: one-hot built via tensor_scalar is_equal at
  4x bf16 (oh from iota_row x dr column; ohT from a PE ones x drT-row
  broadcast + ACT copy); qg via PE; lp/z/wz batched per sub-region; exp and
  PSUM->SBUF copies on the idle Scalar engine; acc via single matmul on
  [wz|ze] (132 cols). alin is inlined per group right after its acc stops.
- Softmax denominator accumulated as extra 4 columns of the acc matmul.
"""
import math
import os
import sys

import numpy as np
import ml_dtypes

sys.path.insert(0, "/opt/trn_rl_repo")

BF = ml_dtypes.bfloat16
H, D, C, L = 4, 32, 128, 2
INV_SQRT_D = 1.0 / math.sqrt(D)
P = 128
NCORES = 8
SHARD = 5120
NGRP = SHARD // P          # 40
NPAD = NCORES * SHARD      # 40960
NREAL = 40000
W1_BASE = 8192             # window1 = rows [8192, 40960)
W0_LIM = 32768             # window0 = rows [0, 32768)
LO_CAP = 4                 # target lo blocks / group
HI_CAP = 3                 # target hi blocks / group
GRP_PER_CHUNK = 4

LAST_RESULT = None


# ---------------------------------------------------------------- host prep
def _perm_for_type(dst_deg):
    """Per-core LPT assignment of nodes to 40 groups by dst degree.
    Returns new_of_old positions [NPAD]."""
    new_of_old = np.empty(NPAD, np.int64)
    for c in range(NCORES):
        ids = np.arange(c * SHARD, (c + 1) * SHARD)
        deg = dst_deg[ids]
        order = np.argsort(-deg, kind="stable")
        gcount = np.zeros(NGRP, np.int64)
        gload = np.zeros(NGRP, np.int64)
        grp_of = np.empty(SHARD, np.int64)
        slot_of = np.empty(SHARD, np.int64)
        for i in order:
            masked = np.where(gcount < P, gload, np.iinfo(np.int64).max)
            g = int(np.argmin(masked))
            grp_of[i] = g
            slot_of[i] = gcount[g]
            gcount[g] += 1
            gload[g] += deg[i]
        new_of_old[ids] = c * SHARD + grp_of * P + slot_of
    return new_of_old


def _prep_edges(edge, perm_src, perm_dst):
    """Partition one relation's edges by dst shard into (lo, hi) window
    blocks per 128-dst group. Returns per-core idx/dr/drT arrays plus the
    static schedule (nblk_lo, nblk_hi per group)."""
    src = perm_src[np.asarray(edge[0]).astype(np.int64)]
    dst = perm_dst[np.asarray(edge[1]).astype(np.int64)]
    core = dst // SHARD
    # first pass: per (core, group) counts to fix the static schedule
    lists = {}
    nblk_lo = np.full(NGRP, LO_CAP, np.int64)
    nblk_hi = np.full(NGRP, HI_CAP, np.int64)
    for c in range(NCORES):
        m = core == c
        s, dl = src[m], dst[m] - c * SHARD
        g = dl // P
        rel = dl % P
        for gi in range(NGRP):
            gm = g == gi
            sg, rg = s[gm], rel[gm]
            lo_forced = sg < W1_BASE
            hi_forced = sg >= W0_LIM
            flex = ~lo_forced & ~hi_forced
            nlo_f = int(lo_forced.sum())
            cap = LO_CAP * P
            take = min(int(flex.sum()), max(0, cap - nlo_f))
            # order: lo-forced, then `take` flex edges into lo; rest to hi
            fidx = np.where(flex)[0]
            lo_idx = np.concatenate([np.where(lo_forced)[0], fidx[:take]])
            hi_idx = np.concatenate([fidx[take:], np.where(hi_forced)[0]])
            lists[(c, gi)] = (sg[lo_idx], rg[lo_idx], sg[hi_idx] - W1_BASE,
                              rg[hi_idx])
            nblk_lo[gi] = max(nblk_lo[gi], (len(lo_idx) + P - 1) // P)
            nblk_hi[gi] = max(nblk_hi[gi], (len(hi_idx) + P - 1) // P)
    TB_lo = int(nblk_lo.sum())
    TB_hi = int(nblk_hi.sum())
    TB = TB_lo + TB_hi
    SL = TB * P
    gmax = int((nblk_lo + nblk_hi).max())
    idx_ws, dr_ws, drT_ws = [], [], []
    for c in range(NCORES):
        idx = np.zeros(SL, np.int16)
        dr = np.full(SL, -1.0, np.float32)
        pos = 0
        for gi in range(NGRP):
            sl_, rl_, _, _ = lists[(c, gi)]
            idx[pos:pos + len(sl_)] = sl_
            dr[pos:pos + len(sl_)] = rl_
            pos += int(nblk_lo[gi]) * P
        for gi in range(NGRP):
            _, _, sh_, rh_ = lists[(c, gi)]
            idx[pos:pos + len(sh_)] = sh_
            dr[pos:pos + len(sh_)] = rh_
            pos += int(nblk_hi[gi]) * P
        assert pos == SL
        idx_ws.append(np.tile(idx.reshape(SL // 16, 16).T, (8, 1)).copy())
        dr_ws.append(dr.reshape(TB, P).T.copy())
        drT_ws.append(dr.reshape(1, SL).astype(BF).copy())
    meta = (nblk_lo.tolist(), nblk_hi.tolist(), TB_lo, TB_hi, gmax)
    return idx_ws, dr_ws, drT_ws, meta


def _chunks_of(nblk, grp_per_chunk=GRP_PER_CHUNK):
    """Chunk = gather for `grp_per_chunk` consecutive groups.
    Returns (chunks [(blk0, nblk)], per-group (chunk_id, off_in_chunk))."""
    chunks, gloc = [], []
    b = 0
    for g0 in range(0, NGRP, grp_per_chunk):
        n = sum(nblk[g0:g0 + grp_per_chunk])
        ci = len(chunks)
        off = 0
        for g in range(g0, min(g0 + grp_per_chunk, NGRP)):
            gloc.append((ci, off))
            off += nblk[g]
        chunks.append((b, n))
        b += n
    return chunks, gloc


def _fold_weights(ins):
    """Fold a_rel/m_rel into k/v, p_rel/sqrt(D) into q, sigmoid(skip) into
    a_lin. All matmul operands as bf16; biases may be None when zero."""
    f = {}
    for l in range(L):
        for t in range(2):
            kw = np.asarray(ins["k_w"][l, t], np.float32)
            kb = np.asarray(ins["k_b"][l, t], np.float32)
            vw = np.asarray(ins["v_w"][l, t], np.float32)
            vb = np.asarray(ins["v_b"][l, t], np.float32)
            ar = np.asarray(ins["a_rel"][l, t], np.float32)
            mr = np.asarray(ins["m_rel"][l, t], np.float32)
            wk = np.zeros((C, C), np.float32)
            wv = np.zeros((C, C), np.float32)
            bk = np.zeros(C, np.float32)
            bv = np.zeros(C, np.float32)
            for h in range(H):
                sl = slice(h * D, (h + 1) * D)
                wk[:, sl] = kw[:, sl] @ ar[h]
                wv[:, sl] = vw[:, sl] @ mr[h]
                bk[sl] = kb[sl] @ ar[h]
                bv[sl] = vb[sl] @ mr[h]
            r_dst = 1 - t
            pr = np.asarray(ins["p_rel"][l, r_dst], np.float32) * INV_SQRT_D
            scale = np.repeat(pr, D)
            wq = np.asarray(ins["q_w"][l, t], np.float32) * scale[None, :]
            bq = np.asarray(ins["q_b"][l, t], np.float32) * scale
            f[f"Wkvq{l}{t}"] = np.concatenate([wk, wv, wq], 1).astype(BF)
            bkvq = np.concatenate([bk, bv, bq])
            f[f"bkvq{l}{t}"] = (bkvq.reshape(1, 3 * C).astype(BF)
                                if np.any(bkvq) else None)
            s = 1.0 / (1.0 + math.exp(-float(np.asarray(ins["skip"][l, t]))))
            f[f"Wal{l}{t}"] = (np.asarray(ins["a_lin_w"][l, t], np.float32)
                               * s).astype(BF)
            bal = np.asarray(ins["a_lin_b"][l, t], np.float32) * s
            f[f"bal{l}{t}"] = (bal.reshape(1, C).astype(BF)
                               if np.any(bal) else None)
            f[f"oms{l}{t}"] = 1.0 - s
    f["Wina"] = np.asarray(ins["lin_a_w"], np.float32).astype(BF)
    f["Winb"] = np.asarray(ins["lin_b_w"], np.float32).astype(BF)
    for t, n in enumerate(["lin_a_b", "lin_b_b"]):
        b = np.asarray(ins[n], np.float32)
        f[f"bin{t}"] = b.reshape(C, 1).copy() if np.any(b) else None
    return f


# ------------------------------------------------------------------ kernel
def kernel(**ins):
    global LAST_RESULT
    import concourse.bass as bass
    import concourse.tile as tile
    from concourse import bacc, mybir
    from concourse.bass_utils import run_bass_kernel_spmd
    from concourse.masks import make_identity

    FP = mybir.dt.float32
    BF16 = mybir.dt.bfloat16
    I16 = mybir.dt.int16
    I32 = mybir.dt.int32
    AL = mybir.AluOpType
    AF = mybir.ActivationFunctionType

    f = _fold_weights(ins)

    # dst-degree balancing permutations (type t is dst of rel 1-t)
    edges_np = [np.asarray(ins["edge_ab"]), np.asarray(ins["edge_ba"])]
    deg = [np.zeros(NPAD, np.int64), np.zeros(NPAD, np.int64)]
    np.add.at(deg[1], edges_np[0][1], 1)   # edge_ab dst = type b
    np.add.at(deg[0], edges_np[1][1], 1)   # edge_ba dst = type a
    perm = [_perm_for_type(deg[0]), _perm_for_type(deg[1])]

    # rel r: src type r, dst type 1-r
    idx0, dr0, drT0, meta0 = _prep_edges(edges_np[0], perm[0], perm[1])
    idx1, dr1, drT1, meta1 = _prep_edges(edges_np[1], perm[1], perm[0])
    rel_idx, rel_dr, rel_drT = [idx0, idx1], [dr0, dr1], [drT0, drT1]
    rel_meta = [meta0, meta1]
    TBs = [meta0[2] + meta0[3], meta1[2] + meta1[3]]

    # permuted transposed inputs, bf16, per-core shard slices
    xs = []
    for t, name in enumerate(["x_a", "x_b"]):
        x = np.asarray(ins[name], np.float32)
        xp = np.zeros((NPAD, x.shape[1]), np.float32)
        xp[perm[t][:NREAL]] = x
        xs.append(np.ascontiguousarray(xp.T).astype(BF))
    DA, DB = xs[0].shape[0], xs[1].shape[0]

    nc = bacc.Bacc("TRN2", target_bir_lowering=False, debug=False,
                   num_devices=NCORES)

    # ---- DRAM tensors ----
    t_xs = [nc.dram_tensor("xsa", [DA, SHARD], BF16, kind="ExternalInput").ap(),
            nc.dram_tensor("xsb", [DB, SHARD], BF16, kind="ExternalInput").ap()]
    wnames = [n for n in f if f[n] is not None
              and isinstance(f[n], np.ndarray)]
    t_w = {}
    for n in wnames:
        dt = FP if f[n].dtype == np.float32 else BF16
        t_w[n] = nc.dram_tensor(n, list(f[n].shape), dt,
                                kind="ExternalInput").ap()
    t_idx = [nc.dram_tensor(f"idx{r}", [P, TBs[r] * 8], I16,
                            kind="ExternalInput").ap() for r in range(2)]
    t_dr = [nc.dram_tensor(f"dr{r}", [P, TBs[r]], FP,
                           kind="ExternalInput").ap() for r in range(2)]
    t_drT = [nc.dram_tensor(f"drT{r}", [1, TBs[r] * P], BF16,
                            kind="ExternalInput").ap() for r in range(2)]

    t_kv_own = [[nc.dram_tensor(f"kvown{l}{t}", [SHARD, 2 * C], BF16)
                 for t in range(2)] for l in range(L)]
    t_kv_full = [[nc.dram_tensor(f"kvfull{l}{t}", [NPAD, 2 * C], BF16,
                                 addr_space="Shared")
                  for t in range(2)] for l in range(L)]
    t_out = [nc.dram_tensor(f"outT{t}", [C, SHARD], FP,
                            kind="ExternalOutput").ap() for t in range(2)]

    with tile.TileContext(nc) as tc:
        cpool_cm = tc.tile_pool(name="const", bufs=1)
        cpool = cpool_cm.__enter__()

        ident = cpool.tile([P, P], BF16)
        make_identity(nc, ident[:])
        ioi = cpool.tile([P, P], I32)
        nc.gpsimd.iota(ioi[:], pattern=[[1, P]], base=0, channel_multiplier=0)
        iota_row = cpool.tile([P, P], BF16)
        nc.vector.tensor_copy(iota_row[:], ioi[:])
        ioc = cpool.tile([P, 1], I32)
        nc.gpsimd.iota(ioc[:], pattern=[[0, 1]], base=0, channel_multiplier=1)
        iota_col = cpool.tile([P, 1], FP)
        nc.vector.tensor_copy(iota_col[:], ioc[:])
        ones_row = cpool.tile([1, P], BF16)
        nc.vector.memset(ones_row[:], 1.0)

        w_sb = {}
        for n in wnames:
            w_sb[n] = cpool.tile(list(f[n].shape),
                                 FP if f[n].dtype == np.float32 else BF16,
                                 name=n, tag=n)
            nc.sync.dma_start(out=w_sb[n][:], in_=t_w[n][:])
        xs_sb = []
        for t in range(2):
            xt = cpool.tile([t_xs[t].shape[0], SHARD], BF16, tag=f"xs{t}")
            nc.sync.dma_start(out=xt[:], in_=t_xs[t][:])
            xs_sb.append(xt)
        idx_sb, dr_sb = [], []
        for r in range(2):
            it = cpool.tile([P, TBs[r] * 8], I16, tag=f"idxsb{r}")
            nc.sync.dma_start(out=it[:], in_=t_idx[r][:])
            idx_sb.append(it)
            drt = cpool.tile([P, TBs[r]], FP, tag=f"drsb{r}")
            nc.sync.dma_start(out=drt[:], in_=t_dr[r][:])
            dr_sb.append(drt)


        xT_sb = [cpool.tile([P, NGRP, P], BF16, name=f"xT{t}", tag=f"xT{t}")
                 for t in range(2)]
        q_sb = [cpool.tile([P, NGRP, C], BF16, name=f"q{t}", tag=f"q{t}")
                for t in range(2)]

        # ---------- stage A: x0T + kv0/q0 shard (per type), AllGather ----
        def kvq_group(t, l, g, lhsT_ap, ps_pool, sb_pool):
            """kv/q projection for group g of type t, layer l, from
            transposed activations lhsT_ap [C, P]. Writes q_sb and DMAs the
            kv rows to t_kv_own[l][t]."""
            ps = ps_pool.tile([P, 3 * C], FP, space="PSUM", tag="kvq")
            bias = f[f"bkvq{l}{t}"] is not None
            nc.tensor.matmul(out=ps[:], lhsT=lhsT_ap,
                             rhs=w_sb[f"Wkvq{l}{t}"][:],
                             start=True, stop=not bias)
            if bias:
                nc.tensor.matmul(out=ps[:], lhsT=ones_row[:],
                                 rhs=w_sb[f"bkvq{l}{t}"][:],
                                 start=False, stop=True)
            kvt = sb_pool.tile([P, 2 * C], BF16, tag="kvt")
            nc.scalar.activation(out=kvt[:], in_=ps[:, 0:2 * C], func=AF.Copy)
            nc.vector.tensor_copy(q_sb[t][:, g, :], ps[:, 2 * C:3 * C])
            nc.sync.dma_start(
                out=t_kv_own[l][t].ap()[g * P:(g + 1) * P, :], in_=kvt[:])

        def allgather_kv(l, t):
            nc.gpsimd.collective_compute(
                "AllGather", mybir.AluOpType.bypass,
                replica_groups=[list(range(NCORES))],
                ins=[t_kv_own[l][t].ap()[:]],
                outs=[t_kv_full[l][t].ap()[:]],
            )

        with (
            tc.tile_pool(name="a_ps", bufs=2, space="PSUM") as ps_pool,
            tc.tile_pool(name="a_sb", bufs=3) as sb_pool,
        ):
            for t in range(2):
                Win = w_sb["Wina" if t == 0 else "Winb"]
                bin_t = f[f"bin{t}"]
                for g in range(NGRP):
                    psx = ps_pool.tile([P, P], FP, space="PSUM", tag="x0T")
                    nc.tensor.matmul(out=psx[:], lhsT=Win[:],
                                     rhs=xs_sb[t][:, g * P:(g + 1) * P],
                                     start=True, stop=True)
                    if bin_t is not None:
                        nc.scalar.activation(out=xT_sb[t][:, g, :],
                                             in_=psx[:], func=AF.Relu,
                                             bias=w_sb[f"bin{t}"][:])
                    else:
                        nc.scalar.activation(out=xT_sb[t][:, g, :],
                                             in_=psx[:], func=AF.Relu)
                for g in range(NGRP):
                    kvq_group(t, 0, g, xT_sb[t][:, g, :], ps_pool, sb_pool)
                allgather_kv(0, t)

        # ---------- attention + inlined alin ----------
        def attention(r, l):
            td = 1 - r           # dst type
            nblk_lo, nblk_hi, TB_lo, TB_hi, gmax = rel_meta[r]
            lo_chunks, lo_loc = _chunks_of(nblk_lo)
            hi_chunks, hi_loc = _chunks_of(nblk_hi)
            max_lo = max(n for _, n in lo_chunks)
            max_hi = max(n for _, n in hi_chunks)
            max_nb = max(max(nblk_lo), max(nblk_hi))
            kv_ap = t_kv_full[l][r].ap()
            idxt, drt = idx_sb[r], dr_sb[r]
            oms = f[f"oms{l}{td}"]
            bal = f[f"bal{l}{td}"] is not None

            with (
                tc.tile_pool(name=f"g{r}{l}", bufs=2) as gpool,
                tc.tile_pool(name=f"ps{r}{l}", bufs=2, space="PSUM") as aps,
                tc.tile_pool(name=f"rps{r}{l}", bufs=1, space="PSUM") as rps,
                tc.tile_pool(name=f"acc{r}{l}", bufs=2, space="PSUM") as accp,
                tc.tile_pool(name=f"alps{r}{l}", bufs=1, space="PSUM") as alp,
                tc.tile_pool(name=f"sb{r}{l}", bufs=2) as asb,
                tc.tile_pool(name=f"al{r}{l}", bufs=2) as alsb,
                tc.tile_pool(name=f"dT{r}{l}", bufs=1) as dTpool,
            ):
                drTt = dTpool.tile([1, (TB_lo + TB_hi) * P], BF16, tag="drTa")
                nc.sync.dma_start(out=drTt[:], in_=t_drT[r][:])
                gt_lo = gt_hi = None

                def gather(region, ci):
                    chunks = lo_chunks if region == 0 else hi_chunks
                    base = 0 if region == 0 else TB_lo
                    b0, n = chunks[ci]
                    mx = max_lo if region == 0 else max_hi
                    gt = gpool.tile([P, mx, 2 * C], BF16,
                                    tag=f"kvch{region}")
                    in_ap = (kv_ap[0:W0_LIM, :] if region == 0
                             else kv_ap[W1_BASE:NPAD, :])
                    nc.gpsimd.dma_gather(
                        out_ap=gt[:, 0:n, :], in_ap=in_ap,
                        idxs_ap=idxt[:, (base + b0) * 8:(base + b0 + n) * 8],
                        num_idxs=n * P, num_idxs_reg=n * P,
                        elem_size=2 * C, single_packet=False,
                    )
                    return gt

                for g in range(NGRP):
                    ci_lo, off_lo = lo_loc[g]
                    ci_hi, off_hi = hi_loc[g]
                    if g % GRP_PER_CHUNK == 0:
                        gt_lo = gather(0, ci_lo)
                        gt_hi = gather(1, ci_hi)
                    acc = accp.tile([P, 160], FP, space="PSUM", tag="acc")
                    subs = [(gt_lo, nblk_lo[g],
                             sum(nblk_lo[:g]), off_lo, 0),
                            (gt_hi, nblk_hi[g],
                             TB_lo + sum(nblk_hi[:g]), off_hi, nblk_lo[g])]
                    nsub_blks = nblk_lo[g] + nblk_hi[g]
                    bdone = 0
                    for gt, nb, gb0, coff, doff in subs:
                        # replicated dst rows for this sub-region
                        rep_ps = rps.tile([P, max_nb * P], FP, space="PSUM",
                                          tag="rep")
                        nc.tensor.matmul(
                            out=rep_ps[:, 0:nb * P], lhsT=ones_row[:],
                            rhs=drTt[:, gb0 * P:(gb0 + nb) * P],
                            start=True, stop=True)
                        rep_sb = asb.tile([P, max_nb * P], BF16, tag="repsb")
                        nc.scalar.activation(out=rep_sb[:, 0:nb * P],
                                             in_=rep_ps[:, 0:nb * P],
                                             func=AF.Copy)
                        oh = asb.tile([P, max_nb, P], BF16, tag="oh")
                        ohT = asb.tile([P, max_nb, P], BF16, tag="ohT")
                        qg_ps = aps.tile([P, max_nb * C], FP, space="PSUM",
                                         tag="qg")
                        for b in range(nb):
                            nc.vector.tensor_scalar(
                                out=oh[:, b, :], in0=iota_row[:],
                                scalar1=drt[:, gb0 + b:gb0 + b + 1],
                                scalar2=None, op0=AL.is_equal)
                            nc.vector.tensor_scalar(
                                out=ohT[:, b, :],
                                in0=rep_sb[:, b * P:(b + 1) * P],
                                scalar1=iota_col[:],
                                scalar2=None, op0=AL.is_equal)
                            nc.tensor.matmul(
                                out=qg_ps[:, b * C:(b + 1) * C],
                                lhsT=ohT[:, b, :], rhs=q_sb[td][:, g, :],
                                start=True, stop=True)
                        qg_sb = asb.tile([P, max_nb * C], BF16, tag="qgsb")
                        nc.scalar.activation(out=qg_sb[:, 0:nb * C],
                                             in_=qg_ps[:, 0:nb * C],
                                             func=AF.Copy)
                        lp = asb.tile([P, max_nb, C], BF16, tag="lp")
                        nc.vector.tensor_tensor(
                            out=lp[:, 0:nb, :],
                            in0=qg_sb[:, 0:nb * C].rearrange(
                                "p (b c) -> p b c", b=nb),
                            in1=gt[:, coff:coff + nb, 0:C], op=AL.mult)
                        z = asb.tile([P, max_nb * H], FP, tag="z")
                        nc.vector.tensor_reduce(
                            out=z[:, 0:nb * H].rearrange(
                                "p (b h) -> p b h", h=H),
                            in_=lp[:, 0:nb, :].rearrange(
                                "p b (h d) -> p b h d", h=H),
                            axis=mybir.AxisListType.X, op=AL.add)
                        wze = asb.tile([P, max_nb, 160], BF16, tag="wze")
                        nc.scalar.activation(
                            out=wze[:, 0:nb, C:C + H],
                            in_=z[:, 0:nb * H].rearrange(
                                "p (b h) -> p b h", h=H),
                            func=AF.Exp)
                        nc.vector.tensor_tensor(
                            out=wze[:, 0:nb, 0:C],
                            in0=gt[:, coff:coff + nb, C:2 * C],
                            in1=wze[:, 0:nb, C:C + H].rearrange(
                                "p b (h o) -> p b h o", o=1).to_broadcast(
                                    [P, nb, H, D]),
                            op=AL.mult)
                        for b in range(nb):
                            nc.tensor.matmul(
                                out=acc[:, 0:C + H], lhsT=oh[:, b, :],
                                rhs=wze[:, b, 0:C + H],
                                start=(bdone + b == 0),
                                stop=(bdone + b == nsub_blks - 1))
                        bdone += nb

                    # ---------- inlined alin for group g ----------
                    den = alsb.tile([P, H], FP, tag="den")
                    nc.vector.tensor_scalar(out=den[:], in0=acc[:, C:C + H],
                                            scalar1=1e-16, scalar2=None,
                                            op0=AL.add)
                    rec = alsb.tile([P, H], FP, tag="rec")
                    nc.vector.reciprocal(rec[:], den[:])
                    at = alsb.tile([P, C], FP, tag="at")
                    nc.vector.tensor_tensor(
                        out=at[:], in0=acc[:, 0:C],
                        in1=rec[:].rearrange(
                            "p (h o) -> p h o", o=1).to_broadcast([P, H, D]),
                        op=AL.mult)
                    gl = alsb.tile([P, C], BF16, tag="gl")
                    nc.scalar.activation(out=gl[:], in_=at[:], func=AF.Gelu)
                    glT_ps = alp.tile([P, P], BF16, space="PSUM", tag="glT")
                    nc.tensor.transpose(out=glT_ps[:], in_=gl[:],
                                        identity=ident[:])
                    glT = alsb.tile([P, P], BF16, tag="glTsb")
                    nc.vector.tensor_copy(glT[:], glT_ps[:])
                    o1_ps = alp.tile([P, P], FP, space="PSUM", tag="o1T")
                    nc.tensor.matmul(out=o1_ps[:], lhsT=w_sb[f"Wal{l}{td}"][:],
                                     rhs=glT[:], start=True, stop=not bal)
                    if bal:
                        nc.tensor.matmul(out=o1_ps[:],
                                         lhsT=w_sb[f"bal{l}{td}"][:],
                                         rhs=ones_row[:],
                                         start=False, stop=True)
                    xsc = alsb.tile([P, P], BF16, tag="xsc")
                    nc.scalar.activation(out=xsc[:], in_=xT_sb[td][:, g, :],
                                         func=AF.Copy, scale=float(oms))
                    if l == 0:
                        nc.vector.tensor_tensor(out=xT_sb[td][:, g, :],
                                                in0=o1_ps[:], in1=xsc[:],
                                                op=AL.add)
                        kvq_group(td, 1, g, xT_sb[td][:, g, :], alp, alsb)
                    else:
                        outw = alsb.tile([P, P], FP, tag="outw")
                        nc.vector.tensor_tensor(out=outw[:], in0=o1_ps[:],
                                                in1=xsc[:], op=AL.add)
                        nc.sync.dma_start(
                            out=t_out[td][:, g * P:(g + 1) * P], in_=outw[:])
                if l == 0:
                    allgather_kv(1, td)

        attention(0, 0)   # dst b ; alin(1,0) inlined ; AG kv1[b]
        attention(1, 0)   # dst a ; alin(0,0) inlined ; AG kv1[a]
        attention(1, 1)   # dst a ; needs kv1[b] ; writes outT[a]
        attention(0, 1)   # dst b ; needs kv1[a] ; writes outT[b]
        cpool_cm.__exit__(None, None, None)

    nc.compile()

    in_maps = []
    for c in range(NCORES):
        m = {"xsa": np.ascontiguousarray(xs[0][:, c * SHARD:(c + 1) * SHARD]),
             "xsb": np.ascontiguousarray(xs[1][:, c * SHARD:(c + 1) * SHARD])}
        for r in range(2):
            m[f"idx{r}"] = rel_idx[r][c]
            m[f"dr{r}"] = rel_dr[r][c]
            m[f"drT{r}"] = rel_drT[r][c]
        for n in wnames:
            m[n] = np.ascontiguousarray(f[n])
        in_maps.append(m)

    res = run_bass_kernel_spmd(
        nc, in_maps, core_ids=list(range(NCORES)),
        trace=bool(os.environ.get("BASS_TRACE")),
    )
    LAST_RESULT = res
    outs = []
    for t in range(2):
        full = np.concatenate(
            [res.results[c][f"outT{t}"] for c in range(NCORES)], axis=1)
        outs.append(np.ascontiguousarray(full.T[perm[t][:NREAL]]))
    return outs[0], outs[1]


# revision 20
# speedup vs baseline: 4.2523x; 1.3600x over previous
"""HGT (2-type, 2-relation, 2-layer) Bass kernel for 8 Trainium2 cores — v2.

Design vs v0 baseline (6.78 ms):
- bf16 for all matmul operands / tables / gathers (PE 4x, gather bytes 2x).
- x kept in TRANSPOSED layout [c, node] end-to-end: input proj emits x0T
  directly (lhsT=Win), no PE transposes outside alin's gelu output.
- kv tables built per-shard only (40 groups/core) and AllGathered in bf16;
  kills the 640-block full-table rebuilds of v0.
- Host-side LPT permutation balances dst-group degree; two overlapping
  int16 gather windows ([0,32K) and [8K,40K)) let flex edges fill lo blocks
  exactly: 4 lo + 3 hi blocks per group (TB=280/rel vs 340).
- Attention per 128-edge block: one-hot built via tensor_scalar is_equal at
  4x bf16 (oh from iota_row x dr column; ohT from a PE ones x drT-row
  broadcast + ACT copy); qg via PE; lp/z/wz batched per sub-region; exp and
  PSUM->SBUF copies on the idle Scalar engine; acc via single matmul on
  [wz|ze] (132 cols). alin is inlined per group right after its acc stops.
- Softmax denominator accumulated as extra 4 columns of the acc matmul.
"""
import math
import os
import sys

import numpy as np
import ml_dtypes

sys.path.insert(0, "/opt/trn_rl_repo")

BF = ml_dtypes.bfloat16
H, D, C, L = 4, 32, 128, 2
INV_SQRT_D = 1.0 / math.sqrt(D)
P = 128
NCORES = 8
SHARD = 5120
NGRP = SHARD // P          # 40
NPAD = NCORES * SHARD      # 40960
NREAL = 40000
W1_BASE = 8192             # window1 = rows [8192, 40960)
W0_LIM = 32768             # window0 = rows [0, 32768)
LO_CAP = 4                 # target lo blocks / group
HI_CAP = 3                 # target hi blocks / group
GRP_PER_CHUNK = 4

LAST_RESULT = None


# ---------------------------------------------------------------- host prep
def _perm_for_type(dst_deg):
    """Per-core LPT assignment of nodes to 40 groups by dst degree.
    Returns new_of_old positions [NPAD]."""
    new_of_old = np.empty(NPAD, np.int64)
    for c in range(NCORES):
        ids = np.arange(c * SHARD, (c + 1) * SHARD)
        deg = dst_deg[ids]
        order = np.argsort(-deg, kind="stable")
        gcount = np.zeros(NGRP, np.int64)
        gload = np.zeros(NGRP, np.int64)
        grp_of = np.empty(SHARD, np.int64)
        slot_of = np.empty(SHARD, np.int64)
        for i in order:
            masked = np.where(gcount < P, gload, np.iinfo(np.int64).max)
            g = int(np.argmin(masked))
            grp_of[i] = g
            slot_of[i] = gcount[g]
            gcount[g] += 1
            gload[g] += deg[i]
        new_of_old[ids] = c * SHARD + grp_of * P + slot_of
    return new_of_old


def _prep_edges(edge, perm_src, perm_dst):
    """Partition one relation's edges by dst shard into (lo, hi) window
    blocks per 128-dst group. Returns per-core idx/dr/drT arrays plus the
    static schedule (nblk_lo, nblk_hi per group)."""
    src = perm_src[np.asarray(edge[0]).astype(np.int64)]
    dst = perm_dst[np.asarray(edge[1]).astype(np.int64)]
    core = dst // SHARD
    # first pass: per (core, group) counts to fix the static schedule
    lists = {}
    nblk_lo = np.full(NGRP, LO_CAP, np.int64)
    nblk_hi = np.full(NGRP, HI_CAP, np.int64)
    for c in range(NCORES):
        m = core == c
        s, dl = src[m], dst[m] - c * SHARD
        g = dl // P
        rel = dl % P
        for gi in range(NGRP):
            gm = g == gi
            sg, rg = s[gm], rel[gm]
            lo_forced = sg < W1_BASE
            hi_forced = sg >= W0_LIM
            flex = ~lo_forced & ~hi_forced
            nlo_f = int(lo_forced.sum())
            cap = LO_CAP * P
            take = min(int(flex.sum()), max(0, cap - nlo_f))
            # order: lo-forced, then `take` flex edges into lo; rest to hi
            fidx = np.where(flex)[0]
            lo_idx = np.concatenate([np.where(lo_forced)[0], fidx[:take]])
            hi_idx = np.concatenate([fidx[take:], np.where(hi_forced)[0]])
            lists[(c, gi)] = (sg[lo_idx], rg[lo_idx], sg[hi_idx] - W1_BASE,
                              rg[hi_idx])
            nblk_lo[gi] = max(nblk_lo[gi], (len(lo_idx) + P - 1) // P)
            nblk_hi[gi] = max(nblk_hi[gi], (len(hi_idx) + P - 1) // P)
    TB_lo = int(nblk_lo.sum())
    TB_hi = int(nblk_hi.sum())
    TB = TB_lo + TB_hi
    SL = TB * P
    gmax = int((nblk_lo + nblk_hi).max())
    idx_ws, dr_ws, drT_ws = [], [], []
    for c in range(NCORES):
        idx = np.zeros(SL, np.int16)
        dr = np.full(SL, -1.0, np.float32)
        pos = 0
        for gi in range(NGRP):
            sl_, rl_, _, _ = lists[(c, gi)]
            idx[pos:pos + len(sl_)] = sl_
            dr[pos:pos + len(sl_)] = rl_
            pos += int(nblk_lo[gi]) * P
        for gi in range(NGRP):
            _, _, sh_, rh_ = lists[(c, gi)]
            idx[pos:pos + len(sh_)] = sh_
            dr[pos:pos + len(sh_)] = rh_
            pos += int(nblk_hi[gi]) * P
        assert pos == SL
        idx_ws.append(np.tile(idx.reshape(SL // 16, 16).T, (8, 1)).copy())
        dr_ws.append(dr.reshape(TB, P).T.copy())
        drT_ws.append(dr.reshape(1, SL).astype(BF).copy())
    meta = (nblk_lo.tolist(), nblk_hi.tolist(), TB_lo, TB_hi, gmax)
    return idx_ws, dr_ws, drT_ws, meta


def _chunks_of(nblk, grp_per_chunk=GRP_PER_CHUNK):
    """Chunk = gather for `grp_per_chunk` consecutive groups.
    Returns (chunks [(blk0, nblk)], per-group (chunk_id, off_in_chunk))."""
    chunks, gloc = [], []
    b = 0
    for g0 in range(0, NGRP, grp_per_chunk):
        n = sum(nblk[g0:g0 + grp_per_chunk])
        ci = len(chunks)
        off = 0
        for g in range(g0, min(g0 + grp_per_chunk, NGRP)):
            gloc.append((ci, off))
            off += nblk[g]
        chunks.append((b, n))
        b += n
    return chunks, gloc


def _fold_weights(ins):
    """Fold a_rel/m_rel into k/v, p_rel/sqrt(D) into q, sigmoid(skip) into
    a_lin. All matmul operands as bf16; biases may be None when zero."""
    f = {}
    for l in range(L):
        for t in range(2):
            kw = np.asarray(ins["k_w"][l, t], np.float32)
            kb = np.asarray(ins["k_b"][l, t], np.float32)
            vw = np.asarray(ins["v_w"][l, t], np.float32)
            vb = np.asarray(ins["v_b"][l, t], np.float32)
            ar = np.asarray(ins["a_rel"][l, t], np.float32)
            mr = np.asarray(ins["m_rel"][l, t], np.float32)
            wk = np.zeros((C, C), np.float32)
            wv = np.zeros((C, C), np.float32)
            bk = np.zeros(C, np.float32)
            bv = np.zeros(C, np.float32)
            for h in range(H):
                sl = slice(h * D, (h + 1) * D)
                wk[:, sl] = kw[:, sl] @ ar[h]
                wv[:, sl] = vw[:, sl] @ mr[h]
                bk[sl] = kb[sl] @ ar[h]
                bv[sl] = vb[sl] @ mr[h]
            r_dst = 1 - t
            pr = np.asarray(ins["p_rel"][l, r_dst], np.float32) * INV_SQRT_D
            scale = np.repeat(pr, D)
            wq = np.asarray(ins["q_w"][l, t], np.float32) * scale[None, :]
            bq = np.asarray(ins["q_b"][l, t], np.float32) * scale
            f[f"Wkvq{l}{t}"] = np.concatenate([wk, wv, wq], 1).astype(BF)
            bkvq = np.concatenate([bk, bv, bq])
            f[f"bkvq{l}{t}"] = (bkvq.reshape(1, 3 * C).astype(BF)
                                if np.any(bkvq) else None)
            s = 1.0 / (1.0 + math.exp(-float(np.asarray(ins["skip"][l, t]))))
            f[f"Wal{l}{t}"] = (np.asarray(ins["a_lin_w"][l, t], np.float32)
                               * s).astype(BF)
            bal = np.asarray(ins["a_lin_b"][l, t], np.float32) * s
            f[f"bal{l}{t}"] = (bal.reshape(1, C).astype(BF)
                               if np.any(bal) else None)
            f[f"oms{l}{t}"] = 1.0 - s
    f["Wina"] = np.asarray(ins["lin_a_w"], np.float32).astype(BF)
    f["Winb"] = np.asarray(ins["lin_b_w"], np.float32).astype(BF)
    for t, n in enumerate(["lin_a_b", "lin_b_b"]):
        b = np.asarray(ins[n], np.float32)
        f[f"bin{t}"] = b.reshape(C, 1).copy() if np.any(b) else None
    return f


# ------------------------------------------------------------------ kernel
def kernel(**ins):
    global LAST_RESULT
    import concourse.bass as bass
    import concourse.tile as tile
    from concourse import bacc, mybir
    from concourse.bass_utils import run_bass_kernel_spmd
    from concourse.masks import make_identity

    FP = mybir.dt.float32
    BF16 = mybir.dt.bfloat16
    I16 = mybir.dt.int16
    I32 = mybir.dt.int32
    AL = mybir.AluOpType
    AF = mybir.ActivationFunctionType

    f = _fold_weights(ins)

    # dst-degree balancing permutations (type t is dst of rel 1-t)
    edges_np = [np.asarray(ins["edge_ab"]), np.asarray(ins["edge_ba"])]
    deg = [np.zeros(NPAD, np.int64), np.zeros(NPAD, np.int64)]
    np.add.at(deg[1], edges_np[0][1], 1)   # edge_ab dst = type b
    np.add.at(deg[0], edges_np[1][1], 1)   # edge_ba dst = type a
    perm = [_perm_for_type(deg[0]), _perm_for_type(deg[1])]

    # rel r: src type r, dst type 1-r
    idx0, dr0, drT0, meta0 = _prep_edges(edges_np[0], perm[0], perm[1])
    idx1, dr1, drT1, meta1 = _prep_edges(edges_np[1], perm[1], perm[0])
    rel_idx, rel_dr, rel_drT = [idx0, idx1], [dr0, dr1], [drT0, drT1]
    rel_meta = [meta0, meta1]
    TBs = [meta0[2] + meta0[3], meta1[2] + meta1[3]]

    # permuted transposed inputs, bf16, per-core shard slices
    xs = []
    for t, name in enumerate(["x_a", "x_b"]):
        x = np.asarray(ins[name], np.float32)
        xp = np.zeros((NPAD, x.shape[1]), np.float32)
        xp[perm[t][:NREAL]] = x
        xs.append(np.ascontiguousarray(xp.T).astype(BF))
    DA, DB = xs[0].shape[0], xs[1].shape[0]

    nc = bacc.Bacc("TRN2", target_bir_lowering=False, debug=False,
                   num_devices=NCORES)

    # ---- DRAM tensors ----
    t_xs = [nc.dram_tensor("xsa", [DA, SHARD], BF16, kind="ExternalInput").ap(),
            nc.dram_tensor("xsb", [DB, SHARD], BF16, kind="ExternalInput").ap()]
    wnames = [n for n in f if f[n] is not None
              and isinstance(f[n], np.ndarray)]
    t_w = {}
    for n in wnames:
        dt = FP if f[n].dtype == np.float32 else BF16
        t_w[n] = nc.dram_tensor(n, list(f[n].shape), dt,
                                kind="ExternalInput").ap()
    t_idx = [nc.dram_tensor(f"idx{r}", [P, TBs[r] * 8], I16,
                            kind="ExternalInput").ap() for r in range(2)]
    t_dr = [nc.dram_tensor(f"dr{r}", [P, TBs[r]], FP,
                           kind="ExternalInput").ap() for r in range(2)]
    t_drT = [nc.dram_tensor(f"drT{r}", [1, TBs[r] * P], BF16,
                            kind="ExternalInput").ap() for r in range(2)]

    t_kv_own = [[nc.dram_tensor(f"kvown{l}{t}", [SHARD, 2 * C], BF16)
                 for t in range(2)] for l in range(L)]
    t_kv_full = [[nc.dram_tensor(f"kvfull{l}{t}", [NPAD, 2 * C], BF16,
                                 addr_space="Shared")
                  for t in range(2)] for l in range(L)]
    t_out = [nc.dram_tensor(f"outT{t}", [C, SHARD], FP,
                            kind="ExternalOutput").ap() for t in range(2)]

    with tile.TileContext(nc) as tc:
        cpool_cm = tc.tile_pool(name="const", bufs=1)
        cpool = cpool_cm.__enter__()

        ident = cpool.tile([P, P], BF16)
        make_identity(nc, ident[:])
        ioi = cpool.tile([P, P], I32)
        nc.gpsimd.iota(ioi[:], pattern=[[1, P]], base=0, channel_multiplier=0)
        iota_row = cpool.tile([P, P], BF16)
        nc.vector.tensor_copy(iota_row[:], ioi[:])
        ioc = cpool.tile([P, 1], I32)
        nc.gpsimd.iota(ioc[:], pattern=[[0, 1]], base=0, channel_multiplier=1)
        iota_col = cpool.tile([P, 1], FP)
        nc.vector.tensor_copy(iota_col[:], ioc[:])
        ones_row = cpool.tile([1, P], BF16)
        nc.vector.memset(ones_row[:], 1.0)

        w_sb = {}
        for n in wnames:
            w_sb[n] = cpool.tile(list(f[n].shape),
                                 FP if f[n].dtype == np.float32 else BF16,
                                 name=n, tag=n)
            nc.sync.dma_start(out=w_sb[n][:], in_=t_w[n][:])
        idx_sb, dr_sb = [], []
        for r in range(2):
            it = cpool.tile([P, TBs[r] * 8], I16, tag=f"idxsb{r}")
            nc.sync.dma_start(out=it[:], in_=t_idx[r][:])
            idx_sb.append(it)
            drt = cpool.tile([P, TBs[r]], FP, tag=f"drsb{r}")
            nc.sync.dma_start(out=drt[:], in_=t_dr[r][:])
            dr_sb.append(drt)


        xT_sb = [cpool.tile([P, NGRP, P], BF16, name=f"xT{t}", tag=f"xT{t}")
                 for t in range(2)]
        q_sb = [cpool.tile([P, NGRP, C], BF16, name=f"q{t}", tag=f"q{t}")
                for t in range(2)]

        # ---------- stage A: x0T + kv0/q0 shard (per type), AllGather ----
        def kvq_group(t, l, g, lhsT_ap, ps_pool, sb_pool):
            """kv/q projection for group g of type t, layer l, from
            transposed activations lhsT_ap [C, P]. Writes q_sb and DMAs the
            kv rows to t_kv_own[l][t]."""
            ps = ps_pool.tile([P, 3 * C], FP, space="PSUM", tag="kvq")
            bias = f[f"bkvq{l}{t}"] is not None
            nc.tensor.matmul(out=ps[:], lhsT=lhsT_ap,
                             rhs=w_sb[f"Wkvq{l}{t}"][:],
                             start=True, stop=not bias)
            if bias:
                nc.tensor.matmul(out=ps[:], lhsT=ones_row[:],
                                 rhs=w_sb[f"bkvq{l}{t}"][:],
                                 start=False, stop=True)
            kvt = sb_pool.tile([P, 2 * C], BF16, tag="kvt")
            nc.scalar.activation(out=kvt[:], in_=ps[:, 0:2 * C], func=AF.Copy)
            nc.scalar.activation(out=q_sb[t][:, g, :], in_=ps[:, 2 * C:3 * C],
                                 func=AF.Copy)
            nc.sync.dma_start(
                out=t_kv_own[l][t].ap()[g * P:(g + 1) * P, :], in_=kvt[:])

        def allgather_kv(l, t):
            nc.gpsimd.collective_compute(
                "AllGather", mybir.AluOpType.bypass,
                replica_groups=[list(range(NCORES))],
                ins=[t_kv_own[l][t].ap()[:]],
                outs=[t_kv_full[l][t].ap()[:]],
            )

        with (
            tc.tile_pool(name="a_ps", bufs=2, space="PSUM") as ps_pool,
            tc.tile_pool(name="a_sb", bufs=3) as sb_pool,
            tc.tile_pool(name="a_xs", bufs=1) as xs_pool,
        ):
            xs_sb = []
            for t in range(2):
                xt = xs_pool.tile([t_xs[t].shape[0], SHARD], BF16,
                                  tag=f"xs{t}")
                nc.sync.dma_start(out=xt[:], in_=t_xs[t][:])
                xs_sb.append(xt)
            for t in range(2):
                Win = w_sb["Wina" if t == 0 else "Winb"]
                bin_t = f[f"bin{t}"]
                for g in range(NGRP):
                    psx = ps_pool.tile([P, P], FP, space="PSUM", tag="x0T")
                    nc.tensor.matmul(out=psx[:], lhsT=Win[:],
                                     rhs=xs_sb[t][:, g * P:(g + 1) * P],
                                     start=True, stop=True)
                    if bin_t is not None:
                        nc.scalar.activation(out=xT_sb[t][:, g, :],
                                             in_=psx[:], func=AF.Relu,
                                             bias=w_sb[f"bin{t}"][:])
                    else:
                        nc.scalar.activation(out=xT_sb[t][:, g, :],
                                             in_=psx[:], func=AF.Relu)
                for g in range(NGRP):
                    kvq_group(t, 0, g, xT_sb[t][:, g, :], ps_pool, sb_pool)
                allgather_kv(0, t)

        # ---------- attention + inlined alin ----------
        def attention(r, l):
            td = 1 - r           # dst type
            nblk_lo, nblk_hi, TB_lo, TB_hi, gmax = rel_meta[r]
            lo_chunks, lo_loc = _chunks_of(nblk_lo)
            hi_chunks, hi_loc = _chunks_of(nblk_hi)
            max_lo = max(n for _, n in lo_chunks)
            max_hi = max(n for _, n in hi_chunks)
            max_nb = max(max(nblk_lo), max(nblk_hi))
            kv_ap = t_kv_full[l][r].ap()
            idxt, drt = idx_sb[r], dr_sb[r]
            oms = f[f"oms{l}{td}"]
            bal = f[f"bal{l}{td}"] is not None

            with (
                tc.tile_pool(name=f"g{r}{l}", bufs=2) as gpool,
                tc.tile_pool(name=f"ps{r}{l}", bufs=2, space="PSUM") as aps,
                tc.tile_pool(name=f"rps{r}{l}", bufs=1, space="PSUM") as rps,
                tc.tile_pool(name=f"acc{r}{l}", bufs=2, space="PSUM") as accp,
                tc.tile_pool(name=f"alps{r}{l}", bufs=1, space="PSUM") as alp,
                tc.tile_pool(name=f"sb{r}{l}", bufs=2) as asb,
                tc.tile_pool(name=f"al{r}{l}", bufs=2) as alsb,
                tc.tile_pool(name=f"dT{r}{l}", bufs=1) as dTpool,
            ):
                drTt = dTpool.tile([1, (TB_lo + TB_hi) * P], BF16, tag="drTa")
                nc.sync.dma_start(out=drTt[:], in_=t_drT[r][:])
                gt_lo = gt_hi = None

                def gather(region, ci):
                    chunks = lo_chunks if region == 0 else hi_chunks
                    base = 0 if region == 0 else TB_lo
                    b0, n = chunks[ci]
                    mx = max_lo if region == 0 else max_hi
                    gt = gpool.tile([P, mx, 2 * C], BF16,
                                    tag=f"kvch{region}")
                    in_ap = (kv_ap[0:W0_LIM, :] if region == 0
                             else kv_ap[W1_BASE:NPAD, :])
                    nc.gpsimd.dma_gather(
                        out_ap=gt[:, 0:n, :], in_ap=in_ap,
                        idxs_ap=idxt[:, (base + b0) * 8:(base + b0 + n) * 8],
                        num_idxs=n * P, num_idxs_reg=n * P,
                        elem_size=2 * C, single_packet=False,
                    )
                    return gt

                GQ = GRP_PER_CHUNK
                gmax = max(nblk_lo[g] + nblk_hi[g] for g in range(NGRP))
                acc_sb = None
                for g in range(NGRP):
                    ci_lo, off_lo = lo_loc[g]
                    ci_hi, off_hi = hi_loc[g]
                    gq = g % GQ
                    if gq == 0:
                        gt_lo = gather(0, ci_lo)
                        gt_hi = gather(1, ci_hi)
                        acc_sb = alsb.tile([P, GQ, 160], FP, tag="accsb")
                    acc = accp.tile([P, 160], FP, space="PSUM", tag="acc")
                    subs = [(gt_lo, nblk_lo[g],
                             sum(nblk_lo[:g]), off_lo),
                            (gt_hi, nblk_hi[g],
                             TB_lo + sum(nblk_hi[:g]), off_hi)]
                    nblks = nblk_lo[g] + nblk_hi[g]
                    oh = asb.tile([P, gmax, P], BF16, tag="oh")
                    ohT = asb.tile([P, gmax, P], BF16, tag="ohT")
                    wze = asb.tile([P, gmax, 160], BF16, tag="wze")
                    z_all = asb.tile([P, gmax * H], FP, tag="z")
                    boff = 0
                    for gt, nb, gb0, coff in subs:
                        # replicated dst rows -> transposed one-hot
                        rep_ps = rps.tile([P, max_nb * P], FP, space="PSUM",
                                          tag="rep")
                        nc.tensor.matmul(
                            out=rep_ps[:, 0:nb * P], lhsT=ones_row[:],
                            rhs=drTt[:, gb0 * P:(gb0 + nb) * P],
                            start=True, stop=True)
                        nc.vector.tensor_tensor(
                            out=ohT[:, boff:boff + nb, :],
                            in0=rep_ps[:, 0:nb * P].rearrange(
                                "p (b j) -> p b j", b=nb),
                            in1=iota_col[:].rearrange(
                                "p (a o) -> p a o", a=1).to_broadcast(
                                    [P, nb, P]),
                            op=AL.is_equal)
                        nc.vector.tensor_tensor(
                            out=oh[:, boff:boff + nb, :],
                            in0=drt[:, gb0:gb0 + nb].rearrange(
                                "p (b o) -> p b o", o=1).to_broadcast(
                                    [P, nb, P]),
                            in1=iota_row[:].rearrange(
                                "p (a j) -> p a j", a=1).to_broadcast(
                                    [P, nb, P]),
                            op=AL.is_equal)
                        qg_ps = aps.tile([P, max_nb * C], FP, space="PSUM",
                                         tag="qg")
                        for b in range(nb):
                            nc.tensor.matmul(
                                out=qg_ps[:, b * C:(b + 1) * C],
                                lhsT=ohT[:, boff + b, :],
                                rhs=q_sb[td][:, g, :],
                                start=True, stop=True)
                        lp = asb.tile([P, max_nb, C], FP, tag="lp")
                        nc.vector.tensor_tensor(
                            out=lp[:, 0:nb, :],
                            in0=qg_ps[:, 0:nb * C].rearrange(
                                "p (b c) -> p b c", b=nb),
                            in1=gt[:, coff:coff + nb, 0:C], op=AL.mult)
                        nc.vector.tensor_reduce(
                            out=z_all[:, boff * H:(boff + nb) * H].rearrange(
                                "p (b h) -> p b h", h=H),
                            in_=lp[:, 0:nb, :].rearrange(
                                "p b (h d) -> p b h d", h=H),
                            axis=mybir.AxisListType.X, op=AL.add)
                        boff += nb
                    nc.scalar.activation(
                        out=wze[:, 0:nblks, C:C + H],
                        in_=z_all[:, 0:nblks * H].rearrange(
                            "p (b h) -> p b h", h=H),
                        func=AF.Exp)
                    boff = 0
                    for gt, nb, gb0, coff in subs:
                        nc.vector.tensor_tensor(
                            out=wze[:, boff:boff + nb, 0:C],
                            in0=gt[:, coff:coff + nb, C:2 * C],
                            in1=wze[:, boff:boff + nb, C:C + H].rearrange(
                                "p b (h o) -> p b h o", o=1).to_broadcast(
                                    [P, nb, H, D]),
                            op=AL.mult)
                        boff += nb
                    for b in range(nblks):
                        nc.tensor.matmul(
                            out=acc[:, 0:C + H], lhsT=oh[:, b, :],
                            rhs=wze[:, b, 0:C + H],
                            start=(b == 0), stop=(b == nblks - 1))
                    nc.scalar.activation(out=acc_sb[:, gq, :], in_=acc[:],
                                         func=AF.Copy)

                    if gq != GQ - 1:
                        continue
                    # ---------- batched alin for the quad ----------
                    g0 = g - GQ + 1
                    den = alsb.tile([P, GQ * H], FP, tag="den")
                    nc.vector.tensor_scalar(
                        out=den[:].rearrange("p (b h) -> p b h", h=H),
                        in0=acc_sb[:, :, C:C + H],
                        scalar1=1e-16, scalar2=None, op0=AL.add)
                    rec = alsb.tile([P, GQ * H], FP, tag="rec")
                    nc.vector.reciprocal(rec[:], den[:])
                    at = alsb.tile([P, GQ, C], FP, tag="at")
                    nc.vector.tensor_tensor(
                        out=at[:], in0=acc_sb[:, :, 0:C],
                        in1=rec[:].rearrange(
                            "p (b h o) -> p b h o", h=H, o=1).to_broadcast(
                                [P, GQ, H, D]),
                        op=AL.mult)
                    gl = alsb.tile([P, GQ, C], BF16, tag="gl")
                    nc.scalar.activation(out=gl[:], in_=at[:], func=AF.Gelu)
                    glT_ps = alp.tile([P, GQ * P], BF16, space="PSUM",
                                      tag="glT")
                    for q4 in range(GQ):
                        nc.tensor.transpose(
                            out=glT_ps[:, q4 * P:(q4 + 1) * P],
                            in_=gl[:, q4, :], identity=ident[:])
                    glT = alsb.tile([P, GQ * P], BF16, tag="glTsb")
                    nc.vector.tensor_copy(glT[:], glT_ps[:])
                    o1_ps = alp.tile([P, GQ * P], FP, space="PSUM", tag="o1T")
                    for q4 in range(GQ):
                        nc.tensor.matmul(
                            out=o1_ps[:, q4 * P:(q4 + 1) * P],
                            lhsT=w_sb[f"Wal{l}{td}"][:],
                            rhs=glT[:, q4 * P:(q4 + 1) * P],
                            start=True, stop=not bal)
                        if bal:
                            nc.tensor.matmul(
                                out=o1_ps[:, q4 * P:(q4 + 1) * P],
                                lhsT=w_sb[f"bal{l}{td}"][:], rhs=ones_row[:],
                                start=False, stop=True)
                    xsc = alsb.tile([P, GQ, P], BF16, tag="xsc")
                    nc.scalar.activation(out=xsc[:],
                                         in_=xT_sb[td][:, g0:g0 + GQ, :],
                                         func=AF.Copy, scale=float(oms))
                    if l == 0:
                        nc.vector.tensor_tensor(
                            out=xT_sb[td][:, g0:g0 + GQ, :],
                            in0=o1_ps[:].rearrange("p (b j) -> p b j", b=GQ),
                            in1=xsc[:], op=AL.add)
                        for q4 in range(GQ):
                            kvq_group(td, 1, g0 + q4,
                                      xT_sb[td][:, g0 + q4, :], alp, alsb)
                    else:
                        outw = alsb.tile([P, GQ, P], FP, tag="outw")
                        nc.vector.tensor_tensor(
                            out=outw[:],
                            in0=o1_ps[:].rearrange("p (b j) -> p b j", b=GQ),
                            in1=xsc[:], op=AL.add)
                        nc.sync.dma_start(
                            out=t_out[td][:, g0 * P:(g0 + GQ) * P],
                            in_=outw[:].rearrange("p b j -> p (b j)"))
                if l == 0:
                    allgather_kv(1, td)

        attention(0, 0)   # dst b ; alin(1,0) inlined ; AG kv1[b]
        attention(1, 0)   # dst a ; alin(0,0) inlined ; AG kv1[a]
        attention(1, 1)   # dst a ; needs kv1[b] ; writes outT[a]
        attention(0, 1)   # dst b ; needs kv1[a] ; writes outT[b]
        cpool_cm.__exit__(None, None, None)

    nc.compile()

    in_maps = []
    for c in range(NCORES):
        m = {"xsa": np.ascontiguousarray(xs[0][:, c * SHARD:(c + 1) * SHARD]),
             "xsb": np.ascontiguousarray(xs[1][:, c * SHARD:(c + 1) * SHARD])}
        for r in range(2):
            m[f"idx{r}"] = rel_idx[r][c]
            m[f"dr{r}"] = rel_dr[r][c]
            m[f"drT{r}"] = rel_drT[r][c]
        for n in wnames:
            m[n] = np.ascontiguousarray(f[n])
        in_maps.append(m)

    res = run_bass_kernel_spmd(
        nc, in_maps, core_ids=list(range(NCORES)),
        trace=bool(os.environ.get("BASS_TRACE")),
    )
    LAST_RESULT = res
    outs = []
    for t in range(2):
        full = np.concatenate(
            [res.results[c][f"outT{t}"] for c in range(NCORES)], axis=1)
        outs.append(np.ascontiguousarray(full.T[perm[t][:NREAL]]))
    return outs[0], outs[1]
